# revision 23
# baseline (speedup 1.0000x reference)
"""Trainium2 Bass kernel for nn_AlignmentModule (conv stems + L2 score +
log-softmax + beta-binomial prior).

Sharding: 8 cores = 4 batches x 2 T_feats halves. Each core computes the
text conv stem for its batch (duplicated within a pair), its half of the
feats conv stem (halo rows come zero-padded from DRAM; the one halo column
of f1 that the reference's conv padding zeroes is masked on device), the
(400, 160) score block with flash-style log-softmax over T_text, adds the
(input-independent, host-precomputed) beta-binomial prior slice, and
writes its output block.

Implementation notes:
 - activations live (C, T) on chip; convs are accumulated PE matmuls over
   shifted windows; conv biases ride as ACT/DVE per-partition bias adds.
 - |f|^2 / |t|^2 via ones-vector matmuls; the distance matrix d2 =
   |f|^2 + |t|^2 - 2 f.t is built in PSUM from 2 K=128 matmuls plus two
   rank-1 augmentation matmuls.
 - all matmul operands are bitcast to float32r (single-pass PE mode,
   1 cycle/row at N>=256); T_text-sized matmuls are padded from 160 to
   256 columns to hit that mode. fro-level error vs fp32 ~1e-4.
 - inputs arrive in 5 packed DMAs; output leaves as one (100, 640) pack.

Self-contained: hardcodes all shapes; reads nothing from disk.
"""

import math
import os
import subprocess
import sys

import numpy as np

import concourse.bass as bass
import concourse.mybir as mybir
import concourse.tile as tile
from concourse.bass_utils import run_bass_kernel_spmd

B, T_TEXT, T_FEATS = 4, 160, 800
ADIM, ODIM = 256, 80
N_CORES = 8
HALF = T_FEATS // 2          # 400 feats rows per core
TT = T_TEXT                  # 160
NP = 256                     # padded T_text matmul width (fp32r full rate)
TTP = NP + 2                 # padded text window width
TFW = HALF + 2               # 402: f1 window  [s-1, s+401)
TFIN = HALF + 4              # 404: feats input window [s-2, s+402)
MT = 100                     # T_feats tile rows per score tile
NMT = HALF // MT             # 4 score tiles
F32 = mybir.dt.float32
F32R = mybir.dt.float32r
MASK_PENALTY = 1.0e12

# biasmask pack layout (128 rows)
BM_BIAS = 0                  # cols 0..15: bias j of ci-chunk c at 8c+j
BM_MASK2 = 16                # cols 16..17: f1 halo-column masks
BM_TMROW = 18                # cols 18..18+NP: x_mask penalty row (row 0)
BM_W = BM_TMROW + NP

FP_FEATS = 0                 # fpack: featsT at 0..404, fw1 after
FP_FW1 = TFIN
FP_W = TFIN + 3 * ADIM

TP_TEXT = 0                  # tpack: textT (2 chunks x TTP), tw1 after
TP_TW1 = 2 * TTP
TP_W = 2 * TTP + 2 * 768

F2P_W = 2 * 768             # f2pack: fw2 only

WD_TW2 = 0                  # wdpack: tw2 | fw3
WD_FW3 = 2 * 256
WD_W = 4 * 256

_nc_cache = None
_prior_cache = None


# ---------------------------------------------------------------- host math
def _prior_f64():
    """f64 fallback replica of reference.beta_binomial_prior."""
    try:
        from scipy.special import gammaln as _gl
    except Exception:
        _gl = np.vectorize(math.lgamma)
    T, N = float(T_FEATS), float(T_TEXT)
    a = np.arange(1, T_FEATS + 1, dtype=np.float64)
    b = T - a + 1.0
    k = np.arange(T_TEXT, dtype=np.float64)[:, None]

    def betaln(x, y):
        return _gl(x) + _gl(y) - _gl(x + y)

    logp = (
        _gl(N + 1.0) - _gl(k + 1.0) - _gl(N - k + 1.0)
        + betaln(k + a, N - k + b) - betaln(a, b)
    )
    return np.asarray(logp.T, dtype=np.float32)


_PRIOR_SRC = """
import os
os.environ["JAX_PLATFORMS"] = "cpu"
import numpy as np
import jax.numpy as jnp
from jax.scipy.special import gammaln

T, N = {T}, {N}
a = 1.0 * jnp.arange(1, T + 1, dtype=jnp.float32)
b = 1.0 * (T - a + 1.0)
k = jnp.arange(N, dtype=jnp.float32)[:, None]
Nf = jnp.float32(N)

def betaln(x, y):
    return gammaln(x) + gammaln(y) - gammaln(x + y)

logp = (gammaln(Nf + 1.0) - gammaln(k + 1.0) - gammaln(Nf - k + 1.0)
        + betaln(k + a, Nf - k + b) - betaln(a, b))
np.save({out!r}, np.asarray(logp.T, dtype=np.float32))
"""


def _beta_binomial_prior():
    """beta_binomial_prior(T_FEATS, T_TEXT), matching the reference's jax
    f32 computation (vanilla XLA-CPU lgamma; the neuron backend produces
    identical values). Computed once in a JAX_PLATFORMS=cpu subprocess,
    cached on disk and in-process."""
    global _prior_cache
    if _prior_cache is not None:
        return _prior_cache
    cache = f"/tmp/_bbprior_{T_FEATS}x{T_TEXT}.npy"
    if not os.path.exists(cache):
        try:
            src = _PRIOR_SRC.format(T=T_FEATS, N=T_TEXT, out=cache)
            subprocess.run([sys.executable, "-c", src], check=True,
                           capture_output=True, timeout=600)
        except Exception:
            pass
    if os.path.exists(cache):
        _prior_cache = np.load(cache).astype(np.float32)
    else:
        _prior_cache = _prior_f64()
    return _prior_cache


# ------------------------------------------------------------- BIR patching
def _split_multiwait(nc):
    """This container's walrus accepts at most one sync wait per
    instruction; move extras onto single-wait NOPs just before."""
    for f in nc.m.functions:
        for bb in f.blocks:
            changed = False
            out = []
            for inst in bb.instructions:
                si = inst.sync_info
                if si is not None and len(si.on_wait) > 1:
                    waits = list(si.on_wait)
                    for j, w in enumerate(waits[:-1]):
                        nop = mybir.InstNoOp(name=f"{inst.name}sw{j}")
                        nop.name = f"{inst.name}sw{j}"
                        nop.engine = inst.engine
                        nop.sync_info = mybir.SyncInfo(on_wait=[w], on_update=[])
                        out.append(nop)
                    inst.sync_info = mybir.SyncInfo(
                        on_wait=[waits[-1]], on_update=list(si.on_update)
                    )
                    changed = True
                out.append(inst)
            if changed:
                bb.instructions = out


# ------------------------------------------------------------ device program
def _build_program():
    global _nc_cache
    if _nc_cache is not None:
        return _nc_cache

    nc = bass.Bass("TRN2", target_bir_lowering=False, debug=False,
                   num_devices=N_CORES, enable_asserts=False)
    AF = mybir.ActivationFunctionType
    AX = mybir.AxisListType
    AL = mybir.AluOpType

    d_bm = nc.dram_tensor("biasmask", [128, BM_W], F32, kind="ExternalInput")
    d_fp = nc.dram_tensor("fpack", [ODIM, FP_W], F32R, kind="ExternalInput")
    d_f2 = nc.dram_tensor("f2pack", [128, F2P_W], F32R, kind="ExternalInput")
    d_tp = nc.dram_tensor("tpack", [128, TP_W], F32R, kind="ExternalInput")
    d_wd = nc.dram_tensor("wdpack", [128, WD_W], F32R, kind="ExternalInput")
    d_onc = nc.dram_tensor("onesc", [128, 1], F32R, kind="ExternalInput")
    d_onr = nc.dram_tensor("onesr", [1, HALF], F32R, kind="ExternalInput")
    d_pr = nc.dram_tensor("prior", [MT, NMT * TT], F32, kind="ExternalInput")
    d_out = nc.dram_tensor("out", [MT, NMT * TT], F32, kind="ExternalOutput")

    TB1, TB2, FB1, FB2, FB3, FB3M2 = range(6)

    with tile.TileContext(nc) as tc:
        with (
            tc.tile_pool(name="dpool", bufs=1) as dpool,
            tc.tile_pool(name="spool", bufs=1) as spool,
            tc.tile_pool(name="epool", bufs=2) as epool,
            tc.tile_pool(name="psum", bufs=3, space="PSUM") as psum,
            tc.tile_pool(name="psum1", bufs=1, space="PSUM") as psum1,
            tc.tile_pool(name="psumd", bufs=3, space="PSUM") as psumd,
            tc.tile_pool(name="psumw", bufs=1, space="PSUM") as psumw,
        ):
            # ---------------- input DMAs (need-ordered) -----------------
            fp = dpool.tile([ODIM, FP_W], F32R, name="fp")
            nc.sync.dma_start(fp[:], d_fp.ap())
            bm = dpool.tile([128, BM_W], F32, name="bm")
            nc.scalar.dma_start(bm[:], d_bm.ap())
            f2w = dpool.tile([128, F2P_W], F32R, name="f2w")
            nc.sync.dma_start(f2w[:], d_f2.ap())
            tp = dpool.tile([128, TP_W], F32R, name="tp")
            nc.scalar.dma_start(tp[:], d_tp.ap())
            wd = dpool.tile([128, WD_W], F32R, name="wd")
            nc.sync.dma_start(wd[:], d_wd.ap())
            ones_col = dpool.tile([128, 1], F32R, name="ones_col")
            nc.gpsimd.dma_start(ones_col[:], d_onc.ap())
            ones_row = dpool.tile([1, HALF], F32R, name="ones_row")
            nc.gpsimd.dma_start(ones_row[:], d_onr.ap())
            prior_sb = dpool.tile([MT, NMT * TT], F32, name="prior_sb")
            nc.scalar.dma_start(prior_sb[:], d_pr.ap())

            def bias(c, j):
                return bm[:, 8 * c + j: 8 * c + j + 1]

            def evac_relu(co, p, bj, out):
                # co=0 on ACT, co=1 on DVE to halve the evacuation wall
                if co == 0:
                    nc.scalar.activation(out[:], p[:], AF.Relu, bias=bias(co, bj))
                else:
                    nc.vector.tensor_scalar(out[:], p[:], bias(co, bj), 0.0,
                                            op0=AL.add, op1=AL.max)

            # warm both ACT table slots while DMAs stream
            scr = spool.tile([1, 2], F32, name="scr")
            nc.scalar.activation(scr[0:1, 0:1], bm[0:1, 0:1], AF.Relu)
            nc.scalar.activation(scr[0:1, 1:2], bm[0:1, 0:1], AF.Sqrt)

            # PE HAM warm-up: dummy matmuls on uninitialized scratch while
            # the input DMAs stream (no data deps, output never read)
            wscr = spool.tile([128, 512], mybir.dt.bfloat16, name="wscr")
            nc.vector.memset(wscr[:], 1.0)
            pwarm = psumw.tile([128, 512], F32, name="pwarm")

            def warm_mms(n, cols=384):
                for _ in range(n):
                    nc.tensor.matmul(pwarm[:, 0:cols], wscr[:, 0:128],
                                     wscr[:, 0:cols],
                                     start=True, stop=True,
                                     skip_group_check=True)

            warm_mms(8)

            # ---------------- feats conv1 -------------------------------
            f1_sb = []
            for co in range(2):
                p = psum.tile([128, TFW], F32, name=f"pf1_{co}", tag="convp")
                for k in range(3):
                    nc.tensor.matmul(
                        p[:],
                        fp[:, FP_FW1 + 256 * k + 128 * co:
                           FP_FW1 + 256 * k + 128 * (co + 1)],
                        fp[:, k:k + TFW],
                        start=(k == 0), stop=(k == 2),
                    )
                f1 = spool.tile([128, TFW], F32R, name=f"f1_{co}")
                evac_relu(co, p, FB1, f1)
                # zero the halo column the reference conv padding zeroes
                nc.vector.tensor_mul(f1[:, 0:1], f1[:, 0:1],
                                     bm[:, BM_MASK2:BM_MASK2 + 1].bitcast(F32R))
                nc.vector.tensor_mul(f1[:, TFW - 1:TFW], f1[:, TFW - 1:TFW],
                                     bm[:, BM_MASK2 + 1:BM_MASK2 + 2].bitcast(F32R))
                f1_sb.append(f1)

            warm_mms(3)

            # ---------------- feats conv2 -------------------------------
            f2_sb = []
            for co in range(2):
                p = psum.tile([128, HALF], F32, name=f"pf2_{co}", tag="convp",
                              padded_shape=[128, TFW])
                n = 0
                for ci in range(2):
                    for k in range(3):
                        nc.tensor.matmul(
                            p[:],
                            f2w[:, 768 * ci + 256 * k + 128 * co:
                                768 * ci + 256 * k + 128 * (co + 1)],
                            f1_sb[ci][:, k:k + HALF],
                            start=(n == 0), stop=(n == 5),
                        )
                        n += 1
                f2 = spool.tile([128, HALF], F32R, name=f"f2_{co}")
                evac_relu(co, p, FB2, f2)
                f2_sb.append(f2)

            # ---------------- text conv1 (fills fc2->fc3 PE gap) --------
            t1_sb = []
            for co in range(2):
                p = psum.tile([128, NP], F32, name=f"pt1_{co}", tag="convp",
                              padded_shape=[128, TFW])
                n = 0
                for ci in range(2):
                    for k in range(3):
                        nc.tensor.matmul(
                            p[:],
                            tp[:, TP_TW1 + 768 * ci + 256 * k + 128 * co:
                               TP_TW1 + 768 * ci + 256 * k + 128 * (co + 1)],
                            tp[:, TTP * ci + k: TTP * ci + k + NP],
                            start=(n == 0), stop=(n == 5),
                        )
                        n += 1
                t1 = spool.tile([128, NP], F32R, name=f"t1_{co}")
                evac_relu(co, p, TB1, t1)
                t1_sb.append(t1)

            # ---------------- feats conv3 -------------------------------
            fneg2_sb, ff_sb = [], []
            for co in range(2):
                p = psum.tile([128, HALF], F32, name=f"pf3_{co}", tag="convp",
                              padded_shape=[128, TFW])
                for ci in range(2):
                    nc.tensor.matmul(
                        p[:],
                        wd[:, WD_FW3 + 256 * ci + 128 * co:
                           WD_FW3 + 256 * ci + 128 * (co + 1)],
                        f2_sb[ci][:],
                        start=(ci == 0), stop=(ci == 1),
                    )
                fneg2 = spool.tile([128, HALF], F32R, name=f"fneg2_{co}")
                ff = spool.tile([128, HALF], F32R, name=f"ff_{co}")
                if co == 0:
                    nc.scalar.activation(fneg2[:], p[:], AF.Identity, scale=-2.0,
                                         bias=bias(co, FB3M2))
                    nc.scalar.activation(ff[:], p[:], AF.Square, bias=bias(co, FB3))
                else:
                    nc.vector.tensor_scalar(fneg2[:], p[:], bias(co, FB3), -2.0,
                                            op0=AL.add, op1=AL.mult)
                    fb = spool.tile([128, HALF], F32, name="fb_1")
                    nc.vector.tensor_scalar_add(fb[:], p[:], bias(co, FB3))
                    nc.vector.tensor_mul(ff[:], fb[:], fb[:])
                fneg2_sb.append(fneg2)
                ff_sb.append(ff)

            # ---------------- text conv2 + norms ------------------------
            t_sb, tt_sb = [], []
            for co in range(2):
                p = psum.tile([128, NP], F32, name=f"pt2_{co}", tag="convp",
                              padded_shape=[128, TFW])
                for ci in range(2):
                    nc.tensor.matmul(
                        p[:],
                        wd[:, WD_TW2 + 256 * ci + 128 * co:
                           WD_TW2 + 256 * ci + 128 * (co + 1)],
                        t1_sb[ci][:],
                        start=(ci == 0), stop=(ci == 1),
                    )
                t = spool.tile([128, NP], F32R, name=f"t_{co}")
                tt = spool.tile([128, NP], F32R, name=f"tt_{co}")
                if co == 0:
                    nc.scalar.activation(t[:], p[:], AF.Identity, bias=bias(co, TB2))
                    nc.scalar.activation(tt[:], p[:], AF.Square, bias=bias(co, TB2))
                else:
                    nc.vector.tensor_scalar_add(t[:], p[:], bias(co, TB2))
                    nc.vector.tensor_mul(tt[:], t[:], t[:])
                t_sb.append(t)
                tt_sb.append(tt)

            # d2 partial: -2 f.t for all tiles (starts as soon as fneg2/t land)
            pd2s = []
            for i in range(NMT):
                lo = i * MT
                pd2 = psumd.tile([MT, NP], F32, name=f"pd2_{i}", tag="d2p")
                nc.tensor.matmul(pd2[:], fneg2_sb[0][:, lo:lo + MT],
                                 t_sb[0][:], start=True, stop=False)
                nc.tensor.matmul(pd2[:], fneg2_sb[1][:, lo:lo + MT],
                                 t_sb[1][:], start=False, stop=False)
                pd2s.append(pd2)

            # norms
            ptn2 = psum1.tile([1, NP], F32, name="ptn2", tag="normp",
                              padded_shape=[1, HALF])
            for ci in range(2):
                nc.tensor.matmul(ptn2[:], ones_col[:], tt_sb[ci][:],
                                 start=(ci == 0), stop=(ci == 1))
            tn2m_row = spool.tile([1, NP], F32R, name="tn2m_row")
            nc.vector.tensor_add(tn2m_row[:], ptn2[:],
                                 bm[0:1, BM_TMROW:BM_TMROW + NP])

            pfn2 = psum1.tile([1, HALF], F32, name="pfn2", tag="normp")
            for ci in range(2):
                nc.tensor.matmul(pfn2[:], ones_col[:], ff_sb[ci][:],
                                 start=(ci == 0), stop=(ci == 1))
            fn2_row = spool.tile([1, HALF], F32R, name="fn2_row")
            nc.vector.tensor_copy(fn2_row[:], pfn2[:])

            # d2 augmentation: |f|^2 and |t|^2 (+mask) rank-1 terms
            for i in range(NMT):
                lo = i * MT
                nc.tensor.matmul(pd2s[i][:], fn2_row[:, lo:lo + MT],
                                 ones_row[:, 0:NP], start=False, stop=False)
                nc.tensor.matmul(pd2s[i][:], ones_row[:, lo:lo + MT],
                                 tn2m_row[:], start=False, stop=True)

            # ---------------- epilogue ----------------------------------
            s_all = epool.tile([MT, NMT * TT], F32, name="s_all", bufs=1)
            m_all = epool.tile([MT, NMT], F32, name="m_all", bufs=1)
            z_all = epool.tile([MT, NMT], F32, name="z_all", bufs=1)
            lz_all = epool.tile([MT, NMT], F32, name="lz_all", bufs=1)
            q_all = epool.tile([MT, NMT * TT], F32, name="q_all", bufs=1)
            o_all = epool.tile([MT, NMT * TT], F32, name="o_all", bufs=1)
            for i in range(NMT):
                nc.scalar.activation(s_all[:, i * TT:(i + 1) * TT],
                                     pd2s[i][:, 0:TT], AF.Sqrt)
            for i in range(NMT):
                blk = slice(i * TT, (i + 1) * TT)
                nc.vector.tensor_reduce(m_all[:, i:i + 1], s_all[:, blk],
                                        axis=AX.X, op=AL.min)
                nc.vector.tensor_sub(q_all[:, blk], prior_sb[:, blk],
                                     s_all[:, blk])
            for i in range(NMT):
                blk = slice(i * TT, (i + 1) * TT)
                p_t = epool.tile([MT, TT], F32, name="p_t")
                nc.scalar.activation(p_t[:], s_all[:, blk], AF.Exp,
                                     scale=-1.0, bias=m_all[:, i:i + 1])
                nc.vector.tensor_reduce(z_all[:, i:i + 1], p_t[:],
                                        axis=AX.X, op=AL.add)
                nc.scalar.activation(lz_all[:, i:i + 1], z_all[:, i:i + 1],
                                     AF.Ln)
                nc.vector.tensor_scalar(o_all[:, blk], q_all[:, blk],
                                        m_all[:, i:i + 1], lz_all[:, i:i + 1],
                                        op0=AL.add, op1=AL.subtract)
            nc.sync.dma_start(d_out.ap(), o_all[:])

    _split_multiwait(nc)
    _nc_cache = nc
    return nc


# ------------------------------------------------------------------ host glue
def _h2(a):
    """(256, X) -> (128, 2X): ci chunk c at columns [c*X, (c+1)*X)."""
    return np.concatenate([a[:128], a[128:]], axis=1)


def _prep_shared(t_w1, t_b1, t_w2, t_b2, f_w1, f_b1, f_w2, f_b2, f_w3, f_b3):
    tw1h = np.asarray(t_w1, np.float32).transpose(1, 2, 0).reshape(ADIM, 3 * ADIM)
    tw2h = np.asarray(t_w2, np.float32)[:, :, 0].T
    fw1h = np.asarray(f_w1, np.float32).transpose(1, 2, 0).reshape(ODIM, 3 * ADIM)
    fw2h = np.asarray(f_w2, np.float32).transpose(1, 2, 0).reshape(ADIM, 3 * ADIM)
    fw3h = np.asarray(f_w3, np.float32)[:, :, 0].T

    f2pack = np.ascontiguousarray(_h2(fw2h))
    wdpack = np.empty((128, WD_W), np.float32)
    wdpack[:, WD_TW2:WD_TW2 + 2 * 256] = _h2(tw2h)
    wdpack[:, WD_FW3:WD_FW3 + 2 * 256] = _h2(fw3h)

    biases = np.zeros((256, 8), np.float32)
    for j, v in enumerate([t_b1, t_b2, f_b1, f_b2, f_b3,
                           -2.0 * np.asarray(f_b3)]):
        biases[:, j] = np.asarray(v, np.float32)

    return {
        "f2pack": f2pack,
        "wdpack": wdpack,
        "onesc": np.ones((128, 1), np.float32),
        "onesr": np.ones((1, HALF), np.float32),
        "tw1h2": _h2(tw1h),
        "fw1h": fw1h,
        "biases2": _h2(biases),          # (128, 16)
    }


def _prep_core_inputs(c, text, feats, x_masks, shared):
    b, h = divmod(c, 2)
    s = h * HALF

    bm = np.zeros((128, BM_W), np.float32)
    bm[:, 0:16] = shared["biases2"]
    bm[:, BM_MASK2] = 0.0 if s - 1 < 0 else 1.0
    bm[:, BM_MASK2 + 1] = 0.0 if s + HALF >= T_FEATS else 1.0
    bm[0, BM_TMROW:BM_TMROW + TT] = MASK_PENALTY * x_masks[b].astype(np.float32)

    fpack = np.zeros((ODIM, FP_W), np.float32)
    lo, hi = max(0, s - 2), min(T_FEATS, s + TFW)
    fpack[:, lo - (s - 2):hi - (s - 2)] = feats[b, lo:hi].T
    fpack[:, FP_FW1:] = shared["fw1h"]

    textT = np.zeros((ADIM, TTP), np.float32)
    textT[:, 1:1 + TT] = text[b].T
    tpack = np.empty((128, TP_W), np.float32)
    tpack[:, TP_TEXT:TP_TEXT + 2 * TTP] = _h2(textT)
    tpack[:, TP_TW1:] = shared["tw1h2"]

    prior = _beta_binomial_prior()[s:s + HALF]               # (400, 160)
    prior_pack = np.ascontiguousarray(
        prior.reshape(NMT, MT, TT).transpose(1, 0, 2).reshape(MT, NMT * TT)
    )

    return {
        "biasmask": bm,
        "fpack": fpack,
        "tpack": tpack,
        "f2pack": shared["f2pack"],
        "wdpack": shared["wdpack"],
        "onesc": shared["onesc"],
        "onesr": shared["onesr"],
        "prior": prior_pack,
    }


def kernel(text, feats, text_lengths, feats_lengths, x_masks,
           t_w1, t_b1, t_w2, t_b2, f_w1, f_b1, f_w2, f_b2, f_w3, f_b3):
    text = np.asarray(text, np.float32)
    feats = np.asarray(feats, np.float32)
    x_masks = np.asarray(x_masks)

    shared = _prep_shared(t_w1, t_b1, t_w2, t_b2,
                          f_w1, f_b1, f_w2, f_b2, f_w3, f_b3)
    nc = _build_program()
    in_maps = [_prep_core_inputs(c, text, feats, x_masks, shared)
               for c in range(N_CORES)]
    res = None
    last_exc = None
    for _attempt in range(3):
        try:
            res = run_bass_kernel_spmd(nc, in_maps,
                                       core_ids=list(range(N_CORES)))
            break
        except Exception as e:   # transient NRT exec-unit flake on cold NEFFs
            last_exc = e
    if res is None:
        raise last_exc

    out = np.empty((B, T_FEATS, T_TEXT), np.float32)
    for c in range(N_CORES):
        b, h = divmod(c, 2)
        blk = res.results[c]["out"].reshape(MT, NMT, TT).transpose(1, 0, 2)
        out[b, h * HALF:(h + 1) * HALF, :] = blk.reshape(HALF, TT)
    return out
